# revision 25
# baseline (speedup 1.0000x reference)
"""Trainium2 Bass kernel for nn_B2BConv1d (Hyena-style back-to-back causal
depthwise convs with gating).

Reference computation (B=2, D=4096, L=2048, channels of x are 3*D interleaved
as c = 3*g + p for stream p in {x1, x2, v}):
    features = causal_dw_conv1d(x, w_proj)          # K=3, per-channel weights
    x1, x2, v = de-interleave(features)             # [B, D, L] each
    z = x2 * v
    z = causal_dw_conv1d(z, repeat(w_short, 16))    # K=7, filter shared per 16ch
    out = x1 * z

Sharding: channels (g in [0, 4096)) split across 8 cores, 512 output channels
per core.  No halo needed (convs are along L, fully local per channel).
The host de-interleaves the 3 streams, casts them to fp16 (halves HBM read
traffic: 12 MiB/core instead of 24 MiB), and precomputes weight tables.

Engine plan per 128-channel x 2048 unit (variant "pc"):
  - TensorE: pf2 = diag-matmul conv3(x2) [3 taps] + pz = conv7(z) [7 taps]
    (10 PE taps instead of the baseline's 13).
  - ScalarE (ACT): tap-2 scaled copies for fv/f1, evacuates pf2/pz PSUM->fp16.
  - VectorE (DVE): PAIR_MAC custom op (2 conv taps in one pass via two
    shifted views) for fv and f1 taps 0-1; tap-2 merge adds and both gate
    multiplies as fp16 2x tensor_tensor.
  - Pool (gpsimd): pad memsets only (walrus rejects TensorScalarPtr on Pool;
    gpsimd tensor_add measured slower than the DVE 2x path).
  - DMA: all loads/stores HWDGE fp16.
Variant "v16" keeps the baseline 13-tap PE structure with fp16 loads.
"""

import numpy as np
from contextlib import ExitStack

B, D, L = 2, 4096, 2048
NCORES = 8
DG = D // NCORES          # 512 output channels per core
CPT = 128                 # channels per partition tile
NT = DG // CPT            # 4 partition tiles per core
K3, K7 = 3, 7
NB = 4                    # PSUM bank tiles per unit
BW = L // NB              # 512 columns per bank tile

import os as _os
DEFAULT_VARIANT = _os.environ.get("KVAR", "pc")

_PROG_CACHE = {}
_PAIR_MAC = None


def _get_pair_mac():
    """Register (once) and return the PAIR_MAC custom DVE op:
    out = in0*s0 + in1*s1 with per-partition scalars s0, s1 — two conv taps
    (two shifted views of the same SBUF tile) in one DVE pass."""
    global _PAIR_MAC
    if _PAIR_MAC is not None:
        return _PAIR_MAC
    import concourse.dve_ops as dve_ops
    from concourse.dve_spec import Spec, Src0, Src1, C0, C1
    from concourse.dve_uop import DveOpSpec
    from concourse.dve_spec import lower

    name = "PAIR_MAC_ANT"
    spec = Spec(
        body=Src0 * C0 + Src1 * C1,
        reference=lambda in0, in1, s0, s1, imm2: (
            in0.astype(np.float32) * s0 + in1.astype(np.float32) * s1),
    )
    # compute the uops sha for this arch so DveOp.compile's pin check passes
    shas = {}
    for ver in ("v3", "v4"):
        opcode = max(dve_ops._SUB_OPCODE_FOR_NAME.values()) + 1
        s = DveOpSpec(name=name, opcode=opcode, uops=lower(spec, ver=ver),
                      rd1_en=True)
        shas[ver] = s.sha(ver)
    op = dve_ops.DveOp(name, spec, subdim=False, uops_sha=shas)
    if name not in dve_ops._SUB_OPCODE_FOR_NAME:
        dve_ops.OPS.append(op)
        dve_ops._SUB_OPCODE_FOR_NAME[name] = (
            max(dve_ops._SUB_OPCODE_FOR_NAME.values()) + 1)
        dve_ops.CUSTOM_DVE_SPECS[name] = spec
    _PAIR_MAC = op
    return op


def build_program(niter=1, variant=DEFAULT_VARIANT, hwloop=False):
    """Build + compile the (SPMD, per-core) Bass program. Same program runs on
    all 8 cores; only the DRAM input contents differ.

    variants:
      "v16"   - fp16 loads, baseline engine split (PE does f2/fv conv3+conv7)
      "pc"    - fp16 loads, PE 10 taps, DVE pair-MACs, Pool merges
      "dma16" - loads + store only (DMA roofline probe)
    """
    import concourse.bacc as bacc
    import concourse.mybir as mybir
    import concourse.tile as tile

    f32 = mybir.dt.float32
    f16 = mybir.dt.float16
    mult = mybir.AluOpType.mult
    add = mybir.AluOpType.add
    Copy = mybir.ActivationFunctionType.Copy

    if variant in ("pc", "pcq"):
        pair_mac = _get_pair_mac()

    nc = bacc.Bacc("TRN2", target_bir_lowering=False, debug=False)

    x1d = nc.dram_tensor("x1", [B, DG, L], f16, kind="ExternalInput")
    x2d = nc.dram_tensor("x2", [B, DG, L], f16, kind="ExternalInput")
    xvd = nc.dram_tensor("xv", [B, DG, L], f16, kind="ExternalInput")
    w1d = nc.dram_tensor("w1", [DG, K3], f32, kind="ExternalInput")
    w2d = nc.dram_tensor("w2", [DG, K3], f32, kind="ExternalInput")
    wvd = nc.dram_tensor("wv", [DG, K3], f32, kind="ExternalInput")
    d2d = nc.dram_tensor("d2", [CPT, NT * K3 * CPT], f16, kind="ExternalInput")
    dvd = nc.dram_tensor("dv", [CPT, NT * K3 * CPT], f16, kind="ExternalInput")
    d7d = nc.dram_tensor("d7", [CPT, NT * K7 * CPT], f16, kind="ExternalInput")
    outd = nc.dram_tensor("out", [B, DG, L], f16, kind="ExternalOutput")

    with tile.TileContext(nc) as tc:
        with ExitStack() as ctx:
            wpool = ctx.enter_context(tc.tile_pool(name="wpool", bufs=1))
            xpool = ctx.enter_context(tc.tile_pool(name="xpool", bufs=2))
            mpool = ctx.enter_context(tc.tile_pool(name="mpool", bufs=2))
            opool = ctx.enter_context(tc.tile_pool(name="opool", bufs=2))
            ppool = ctx.enter_context(
                tc.tile_pool(name="ppool", bufs=2, space="PSUM"))
            ppool3 = ctx.enter_context(
                tc.tile_pool(name="ppool3", bufs=2, space="PSUM"))

            # per-partition tap weights, one [CPT, K3] block per g-tile.
            # (Load only what the variant uses: pc/pcq has no fv/f2 stt or
            # dv-diag matmuls, so dvs/w2s stay unloaded.)
            w1s = wpool.tile([CPT, NT * K3], f32)
            wvs = wpool.tile([CPT, NT * K3], f32)
            for gt in range(NT):
                cs = slice(gt * CPT, (gt + 1) * CPT)
                nc.sync.dma_start(w1s[:, gt * K3:(gt + 1) * K3], w1d[cs, :])
                nc.sync.dma_start(wvs[:, gt * K3:(gt + 1) * K3], wvd[cs, :])
            # diag lhsT weight matrices for the PE convs
            d2s = wpool.tile([CPT, NT * K3 * CPT], f16)
            d7s = wpool.tile([CPT, NT * K7 * CPT], f16)
            nc.sync.dma_start(d2s[:], d2d[:, :])
            nc.sync.dma_start(d7s[:], d7d[:, :])
            if variant not in ("pc", "pcq"):
                dvs = wpool.tile([CPT, NT * K3 * CPT], f16)
                nc.sync.dma_start(dvs[:], dvd[:, :])

            def lhsT(dtile, gt, K, k):
                o = (gt * K + k) * CPT
                return dtile[:, o:o + CPT]

            def load_unit(b, gt):
                cs = slice(gt * CPT, (gt + 1) * CPT)
                xt1 = xpool.tile([CPT, 2 + L], f16, tag="xt1")
                xt2 = xpool.tile([CPT, 2 + L], f16, tag="xt2")
                xtv = xpool.tile([CPT, 2 + L], f16, tag="xtv")
                nc.gpsimd.memset(xt1[:, 0:2], 0.0)
                nc.gpsimd.memset(xt2[:, 0:2], 0.0)
                nc.gpsimd.memset(xtv[:, 0:2], 0.0)
                if variant == "pcq":
                    # spread the three stream loads across three HWDGE
                    # queues (SP / ACT / DVE) for DMA-engine parallelism
                    nc.sync.dma_start(xt1[:, 2:2 + L], x1d[b, cs, :])
                    nc.scalar.dma_start(xt2[:, 2:2 + L], x2d[b, cs, :])
                    nc.vector.dma_start(xtv[:, 2:2 + L], xvd[b, cs, :])
                else:
                    nc.sync.dma_start(xt1[:, 2:2 + L], x1d[b, cs, :])
                    nc.sync.dma_start(xt2[:, 2:2 + L], x2d[b, cs, :])
                    nc.sync.dma_start(xtv[:, 2:2 + L], xvd[b, cs, :])
                return xt1, xt2, xtv

            def one_pass_v16():
                # baseline engine split, fp16 loads. f1 accumulated fp32.
                for b in range(B):
                    for gt in range(NT):
                        cs = slice(gt * CPT, (gt + 1) * CPT)
                        xt1, xt2, xtv = load_unit(b, gt)
                        if variant == "dma16":
                            res = opool.tile([CPT, L], f16, tag="res")
                            nc.scalar.activation(res[:], xt1[:, 2:2 + L], Copy)
                            nc.sync.dma_start(outd[b, cs, :], res[:])
                            continue

                        # f1 path fp32: ACT tap0, DVE taps 1-2.
                        f1 = mpool.tile([CPT, L], f32, tag="f1")
                        nc.scalar.activation(
                            f1[:], xt1[:, 0:L], Copy,
                            scale=w1s[:, gt * K3:gt * K3 + 1])
                        for k in (1, 2):
                            nc.vector.scalar_tensor_tensor(
                                f1[:], xt1[:, k:k + L],
                                w1s[:, gt * K3 + k:gt * K3 + k + 1], f1[:],
                                mult, add)

                        z0 = mpool.tile([CPT, 6 + L], f16, tag="z0")
                        nc.gpsimd.memset(z0[:, 0:6], 0.0)
                        res = opool.tile([CPT, L], f16, tag="res")

                        # software-pipeline by one bank tile
                        pf = {}

                        def conv3s(t):
                            c0 = t * BW
                            pf2 = ppool3.tile([CPT, BW], f32, tag="pf2")
                            pfv = ppool3.tile([CPT, BW], f32, tag="pfv")
                            for k in range(K3):
                                nc.tensor.matmul(
                                    pfv[:], lhsT(dvs, gt, K3, k),
                                    xtv[:, c0 + k:c0 + k + BW],
                                    start=(k == 0), stop=(k == K3 - 1))
                            for k in range(K3):
                                nc.tensor.matmul(
                                    pf2[:], lhsT(d2s, gt, K3, k),
                                    xt2[:, c0 + k:c0 + k + BW],
                                    start=(k == 0), stop=(k == K3 - 1))
                            pf[t] = (pf2, pfv)

                        def zstage(t):
                            c0 = t * BW
                            pf2, pfv = pf.pop(t)
                            fvs = mpool.tile([CPT, BW], f16, tag="fvs")
                            nc.scalar.activation(fvs[:], pfv[:], Copy)
                            nc.vector.tensor_mul(
                                z0[:, 6 + c0:6 + c0 + BW], pf2[:], fvs[:])
                            pz = ppool.tile([CPT, BW], f32, tag="pz")
                            for k in range(K7):
                                nc.tensor.matmul(
                                    pz[:], lhsT(d7s, gt, K7, k),
                                    z0[:, c0 + k:c0 + k + BW],
                                    start=(k == 0), stop=(k == K7 - 1))
                            nc.vector.tensor_mul(
                                res[:, c0:c0 + BW], pz[:],
                                f1[:, c0:c0 + BW])

                        conv3s(0)
                        for t in range(1, NB):
                            conv3s(t)
                            zstage(t - 1)
                        zstage(NB - 1)
                        nc.sync.dma_start(outd[b, cs, :], res[:])

            def one_pass_pc():
                # PE: conv3(x2) + conv7. DVE: PAIR_MAC for fv/f1 taps 0-1 +
                # both gate muls (fp16 2x). Pool: tap-2 merges. ACT: evacs.
                for b in range(B):
                    for gt in range(NT):
                        cs = slice(gt * CPT, (gt + 1) * CPT)
                        xt1, xt2, xtv = load_unit(b, gt)
                        k0 = gt * K3

                        # fv taps 0,1 on DVE (one PAIR_MAC pass); tap 2 as an
                        # ACT scaled copy; Pool tensor_add merges them.
                        fv = mpool.tile([CPT, L], f16, tag="fv")
                        tv = mpool.tile([CPT, L], f16, tag="tv")
                        sv = mpool.tile([CPT, L], f16, tag="sv")
                        nc.vector._custom_dve(
                            pair_mac, out=tv[:],
                            in0=xtv[:, 0:L], in1=xtv[:, 1:1 + L],
                            s0=wvs[:, k0:k0 + 1], s1=wvs[:, k0 + 1:k0 + 2],
                            imm2=0.0)
                        nc.scalar.activation(
                            sv[:], xtv[:, 2:2 + L], Copy,
                            scale=wvs[:, k0 + 2:k0 + 3])
                        nc.vector.tensor_add(fv[:], tv[:], sv[:])

                        # f1 same split
                        f1 = mpool.tile([CPT, L], f16, tag="f1")
                        t1 = mpool.tile([CPT, L], f16, tag="t1")
                        s1 = mpool.tile([CPT, L], f16, tag="s1")
                        nc.vector._custom_dve(
                            pair_mac, out=t1[:],
                            in0=xt1[:, 0:L], in1=xt1[:, 1:1 + L],
                            s0=w1s[:, k0:k0 + 1], s1=w1s[:, k0 + 1:k0 + 2],
                            imm2=0.0)
                        nc.scalar.activation(
                            s1[:], xt1[:, 2:2 + L], Copy,
                            scale=w1s[:, k0 + 2:k0 + 3])
                        nc.vector.tensor_add(f1[:], t1[:], s1[:])

                        z0 = mpool.tile([CPT, 6 + L], f16, tag="z0")
                        nc.gpsimd.memset(z0[:, 0:6], 0.0)
                        res = opool.tile([CPT, L], f16, tag="res")
                        pf = {}

                        def conv3s(t):
                            c0 = t * BW
                            pf2 = ppool3.tile([CPT, BW], f32, tag="pf2")
                            for k in range(K3):
                                nc.tensor.matmul(
                                    pf2[:], lhsT(d2s, gt, K3, k),
                                    xt2[:, c0 + k:c0 + k + BW],
                                    start=(k == 0), stop=(k == K3 - 1))
                            pf[t] = pf2

                        def zstage(t):
                            c0 = t * BW
                            pf2 = pf.pop(t)
                            f2s = mpool.tile([CPT, BW], f16, tag="f2s")
                            nc.scalar.activation(f2s[:], pf2[:], Copy)
                            nc.vector.tensor_mul(
                                z0[:, 6 + c0:6 + c0 + BW], f2s[:],
                                fv[:, c0:c0 + BW])
                            pz = ppool.tile([CPT, BW], f32, tag="pz")
                            for k in range(K7):
                                nc.tensor.matmul(
                                    pz[:], lhsT(d7s, gt, K7, k),
                                    z0[:, c0 + k:c0 + k + BW],
                                    start=(k == 0), stop=(k == K7 - 1))
                            pzs = mpool.tile([CPT, BW], f16, tag="pzs")
                            nc.scalar.activation(pzs[:], pz[:], Copy)
                            nc.vector.tensor_mul(
                                res[:, c0:c0 + BW], pzs[:],
                                f1[:, c0:c0 + BW])

                        conv3s(0)
                        for t in range(1, NB):
                            conv3s(t)
                            zstage(t - 1)
                        zstage(NB - 1)
                        nc.sync.dma_start(outd[b, cs, :], res[:])

            body = {"pc": one_pass_pc, "pcq": one_pass_pc}.get(
                variant, one_pass_v16)
            if hwloop and niter > 1:
                with tc.For_i(0, niter, 1):
                    body()
            else:
                for _ in range(niter):
                    body()

    nc.compile()
    return nc


def get_program(niter=1, variant=DEFAULT_VARIANT, hwloop=False):
    key = ("nc", niter, variant, hwloop)
    if key not in _PROG_CACHE:
        _PROG_CACHE[key] = build_program(niter, variant, hwloop)
    return _PROG_CACHE[key]


def _diag_blocks(w, K):
    """w: [DG, K] fp32 -> [CPT, NT*K*CPT] fp16 with
    out[p, (gt*K+k)*CPT + p] = w[gt*CPT + p, k]."""
    out = np.zeros((CPT, NT * K * CPT), np.float16)
    p = np.arange(CPT)
    for gt in range(NT):
        for k in range(K):
            out[p, (gt * K + k) * CPT + p] = w[gt * CPT:(gt + 1) * CPT,
                                               k].astype(np.float16)
    return out


def make_in_maps(x, w_proj, w_short):
    """Host-side sharding: de-interleave the 3 streams, cast to fp16, slice
    channels across cores; precompute per-channel tap weight tables."""
    x = np.asarray(x)
    w_proj = np.asarray(w_proj, dtype=np.float32)
    w_short = np.asarray(w_short, dtype=np.float32)
    # channel c = 3*g + p  ->  [B, G, 3, L]
    xr = x.reshape(B, D, 3, L).astype(np.float16)
    wp = w_proj[:, 0, :].reshape(D, 3, K3)
    w7_full = np.repeat(w_short[:, 0, :], D // w_short.shape[0], axis=0)
    in_maps = []
    for i in range(NCORES):
        g0, g1 = DG * i, DG * (i + 1)
        in_maps.append({
            "x1": np.ascontiguousarray(xr[:, g0:g1, 0, :]),
            "x2": np.ascontiguousarray(xr[:, g0:g1, 1, :]),
            "xv": np.ascontiguousarray(xr[:, g0:g1, 2, :]),
            "w1": np.ascontiguousarray(wp[g0:g1, 0, :]),
            "w2": np.ascontiguousarray(wp[g0:g1, 1, :]),
            "wv": np.ascontiguousarray(wp[g0:g1, 2, :]),
            "d2": _diag_blocks(wp[g0:g1, 1, :], K3),
            "dv": _diag_blocks(wp[g0:g1, 2, :], K3),
            "d7": _diag_blocks(w7_full[g0:g1, :], K7),
        })
    return in_maps


def kernel(x, w_proj, w_short):
    import os
    from concourse.bass_utils import run_bass_kernel_spmd

    nc = get_program(variant=DEFAULT_VARIANT)
    in_maps = make_in_maps(x, w_proj, w_short)
    try:
        res = run_bass_kernel_spmd(nc, in_maps, core_ids=list(range(NCORES)))
    except ModuleNotFoundError:
        # BASS_TRACE set but this axon client has no NTFF profile hook;
        # rerun with tracing off.
        os.environ["BASS_NEVER_TRACE"] = "1"
        res = run_bass_kernel_spmd(nc, in_maps, core_ids=list(range(NCORES)))
    out = np.concatenate([res.results[i]["out"] for i in range(NCORES)], axis=1)
    return np.ascontiguousarray(out.astype(np.float32))
